# revision 1
# baseline (speedup 1.0000x reference)
"""Causal self-attention with post-softmax decay mask — Trainium2 Bass kernel.

Problem shapes (hardcoded): B=2, T=2048, C=1024, H=16 heads, head_dim=64.
Sharding: 8 cores = (batch b = core//4) x (head group g = core%4, 4 heads each).
Each core computes QKV projection for its 4 heads from x[b], causal
flash-style attention with the decay mask, and a partial output projection
(its heads' 256 features x W_proj rows). Host sums the 4 partials per batch.

Per-core kernel (all matmuls bf16, fp32 PSUM accumulation):
  phase 1: K^T/V/Q^T strips from xT and the weight slice (K,V first so
           attention on early q-tiles overlaps the remaining Q chunks).
  phase 2: per q-tile (128 rows) per head: S = Q K^T (row-packed matmul
           pairs), causal mask folded into the PSUM accumulation as an
           extra matmul (strict-lower ones^T @ -1e30*I), exp via ScalarE
           (scale=1/8) with accum_out giving row sums Z.  Softmax
           normalization is DEFERRED to the tiny y^T tile: pieces flow
           exp -> x decay (DVE tensor_tensor in 2x mode; Toeplitz decay
           tile read via negative-stride AP) -> XBAR transpose -> PV
           accumulation, with no dependency on Z.  Z is transposed via a
           PE transpose ([128,4] -> [4,128]), reciprocated, broadcast to
           a [128,128] tile with a K=4 selector matmul, and applied when
           copying y^T out of PSUM (tensor_tensor mult).  Then projection
           + output DMA.  PSUM->SBUF copies ride on Pool; ScalarE does
           only exp.
"""

import math
import sys

sys.path.insert(0, "/opt/trn_rl_repo")

import numpy as np
import ml_dtypes

B, T, C = 2, 2048, 1024
N_HEAD = 16
HD = 64
HEADS_PER_CORE = 4
N_CORES = 8
NQ = T // 128  # 16 q-tiles

BF16 = ml_dtypes.bfloat16


def _decay_values_np(n):
    """decay_values[i] = decay at distance i, faithful to reference (fp64)."""
    dl = 2048 - 16 + 1
    nums = np.linspace(0.0, 1.0, dl, dtype=np.float64)
    decay_values = 1.0 - np.power(nums, 1.0 / np.e)
    return np.concatenate([np.ones(15), decay_values])[:n]


def _decay_matrix_np(n):
    """tril decay matrix, faithful to reference.decay_weight_matrix (fp32)."""
    decay_values = _decay_values_np(n)
    idx = np.arange(n)[:, None] - np.arange(n)[None, :]
    mat = decay_values[np.clip(idx, 0, n - 1)]
    return np.where(idx >= 0, mat, 0.0).astype(np.float32)


def build_nc(T_=T):
    import concourse.bass as bass
    import concourse.bacc as bacc
    import concourse.mybir as mybir
    import concourse.tile as tile

    fp32 = mybir.dt.float32
    bf16 = mybir.dt.bfloat16
    Alu = mybir.AluOpType
    Act = mybir.ActivationFunctionType

    NQ_ = T_ // 128
    nc = bacc.Bacc("TRN2")

    xT = nc.declare_dram_parameter("xT", [C, T_], bf16, isOutput=False)
    # wqkv columns: [q01(128) q23(128) k01(128) k23(128) v0..v3(256)]
    wqkv = nc.declare_dram_parameter("wqkv", [C, 768], bf16, isOutput=False)
    # wp rows: h0 feats(64), h1, h2, h3
    wp = nc.declare_dram_parameter("wp", [256, C], bf16, isOutput=False)
    # Toeplitz decay: dd[qi, m] = d(qi + m - 127); block (tq, k-piece) reads
    # dd columns with stride -1 (see the decay tensor_tensor below).
    dd = nc.declare_dram_parameter("dd", [128, T_], bf16, isOutput=False)
    # causal mask as a rank-128 matmul: maskA strict-lower ones, negI =
    # -1e30 * I; S_psum += maskA^T @ negI puts -1e30 above the diagonal.
    maskA = nc.declare_dram_parameter("maskA", [128, 128], bf16, isOutput=False)
    negI = nc.declare_dram_parameter("negI", [128, 128], bf16, isOutput=False)
    # fp32 identity for the PE transpose of Z ([128,4] -> [4,128])
    ident = nc.declare_dram_parameter("ident", [128, 128], fp32, isOutput=False)
    # selector for broadcasting rzT rows to head-halves: sel[j, 128p + m]
    # = 1 iff j == 2p + m//64
    sel = nc.declare_dram_parameter("sel", [4, 256], bf16, isOutput=False)
    out = nc.declare_dram_parameter("out", [T_, C], fp32, isOutput=True)

    with tile.TileContext(nc) as tc:
        with (
            tc.tile_pool(name="const", bufs=1) as const_pool,
            tc.tile_pool(name="qkvout", bufs=1) as qkv_pool,
            tc.tile_pool(name="p", bufs=8) as p_pool,
            tc.tile_pool(name="pdt", bufs=8) as pdt_pool,
            tc.tile_pool(name="z", bufs=8) as z_pool,
            tc.tile_pool(name="outs", bufs=3) as out_pool,
            tc.tile_pool(name="ps_s", bufs=3, space="PSUM") as ps_s,
            tc.tile_pool(name="ps_y", bufs=2, space="PSUM") as ps_y_pool,
        ):
            # ---- load constants / inputs ----
            # xt/wqkv stream in kc-chunks so the first QKV matmul can start
            # after chunk 0 lands instead of after the full 6 MiB load.
            xt_sb = const_pool.tile([128, 8, T_], bf16)
            wqkv_sb = const_pool.tile([128, 8, 768], bf16)
            xT_r = xT.rearrange("(kc p) t -> p kc t", p=128)
            wqkv_r = wqkv.rearrange("(kc p) m -> p kc m", p=128)
            for kc in range(8):
                nc.sync.dma_start(
                    out=wqkv_sb[:, kc : kc + 1, :], in_=wqkv_r[:, kc : kc + 1, :]
                )
                nc.sync.dma_start(
                    out=xt_sb[:, kc : kc + 1, :], in_=xT_r[:, kc : kc + 1, :]
                )
            wp_sb = const_pool.tile([128, 2, C], bf16)
            nc.sync.dma_start(out=wp_sb, in_=wp.rearrange("(pr p) n -> p pr n", p=128))
            dd_sb = const_pool.tile([128, T_], bf16)
            nc.sync.dma_start(out=dd_sb, in_=dd[:, :])
            maskA_sb = const_pool.tile([128, 128], bf16)
            nc.sync.dma_start(out=maskA_sb, in_=maskA[:, :])
            negI_sb = const_pool.tile([128, 128], bf16)
            nc.sync.dma_start(out=negI_sb, in_=negI[:, :])
            ident_sb = const_pool.tile([128, 128], fp32)
            nc.sync.dma_start(out=ident_sb, in_=ident[:, :])
            sel_sb = const_pool.tile([4, 256], bf16)
            nc.sync.dma_start(out=sel_sb, in_=sel[:, :])

            qt_sb = qkv_pool.tile([128, 2, T_], bf16)  # [feat(2x64), pair, q]
            kt_sb = qkv_pool.tile([128, 2, T_], bf16)
            v_sb = qkv_pool.tile([128, T_ // 128, 256], bf16)  # [k-rows, kc, 4 heads]
            yt_sb = qkv_pool.tile([128, 2, T_], bf16)  # y^T strips per pair

            # ---- phase 1: QKV projections (K and V first, Q streams after
            # so early attention tiles overlap the remaining Q chunks) ----
            def qk_strip(which, dst, nqc):
                cw = min(512, T_ - 512 * nqc)
                for pair in range(2):
                    wcol = 256 * which + 128 * pair
                    ps_full = ps_s.tile([128, 1024], fp32, tag="s")
                    ps = ps_full[:, 0:512]
                    for kc in range(8):
                        nc.tensor.matmul(
                            ps[:, 0:cw],
                            lhsT=wqkv_sb[:, kc, wcol : wcol + 128],
                            rhs=xt_sb[:, kc, 512 * nqc : 512 * nqc + cw],
                            start=(kc == 0),
                            stop=(kc == 7),
                        )
                    cp = nc.vector.tensor_copy if (nqc + pair) % 2 == 0 else nc.scalar.copy
                    cp(out=dst[:, pair, 512 * nqc : 512 * nqc + cw], in_=ps[:, 0:cw])

            def v_tile(tc16):
                ps_full = ps_s.tile([128, 1024], fp32, tag="s")
                ps = ps_full[:, 0:512]
                for kc in range(8):
                    nc.tensor.matmul(
                        ps[:, 0:256],
                        lhsT=xt_sb[:, kc, 128 * tc16 : 128 * tc16 + 128],
                        rhs=wqkv_sb[:, kc, 512:768],
                        start=(kc == 0),
                        stop=(kc == 7),
                    )
                cp = nc.vector.tensor_copy if tc16 % 2 == 0 else nc.scalar.copy
                cp(out=v_sb[:, tc16, :], in_=ps[:, 0:256])

            # ---- phase 2: attention per q-tile ----
            def attention_tile(tq):
                L = 128 * (tq + 1)
                # z4[:, h] = row sums of exp for head h (this q-tile)
                z4 = z_pool.tile([128, 4], fp32, tag="z4")
                npieces = (L + 1023) // 1024
                # [0:128) y^T, [128:256) Mbc, [256:384) zT (pair0 only)
                ps_y_a = ps_y_pool.tile([128, 512], fp32, tag="y")
                ps_y_b = ps_y_pool.tile([128, 512], fp32, tag="y")
                ps_y_t = [ps_y_a, ps_y_b]

                def s_stage(head):
                    """S matmuls + mask + exp + decay + transpose issue."""
                    pair, hin = head // 2, head % 2
                    prow = 64 * hin
                    p_t = p_pool.tile([128, T_], bf16, tag="p")
                    zparts = z_pool.tile([128, 2], fp32, tag="zp")
                    pdt = pdt_pool.tile([128, NQ_, 128], bf16, tag="pdt")
                    for piece in range(npieces):
                        p0 = 1024 * piece
                        pl = min(1024, L - p0)
                        ps = ps_s.tile([128, 1024], fp32, tag="s")
                        nsc = (pl + 511) // 512
                        for sc in range(nsc):
                            scl = min(512, pl - 512 * sc)
                            k0 = p0 + 512 * sc
                            nc.tensor.matmul(
                                ps[:, 512 * sc : 512 * sc + scl],
                                lhsT=qt_sb[
                                    prow : prow + 64,
                                    pair,
                                    128 * tq : 128 * tq + 128,
                                ],
                                rhs=kt_sb[prow : prow + 64, pair, k0 : k0 + scl],
                                start=True,
                                stop=not (sc == nsc - 1 and p0 + pl == L),
                                tile_position=(prow, 0),
                            )
                        if p0 + pl == L:  # diagonal chunk lives here:
                            # accumulate the causal mask into PSUM via
                            # matmul: ps[:, diag] += maskA^T @ negI.
                            off = (L - 128) - p0
                            nc.tensor.matmul(
                                ps[:, off : off + 128],
                                lhsT=maskA_sb,
                                rhs=negI_sb,
                                start=False,
                                stop=True,
                            )
                        nc.scalar.activation(
                            out=p_t[:, p0 : p0 + pl],
                            in_=ps[:, 0:pl],
                            func=Act.Exp,
                            scale=0.125,
                            accum_out=(
                                z4[:, head : head + 1]
                                if npieces == 1
                                else zparts[:, piece : piece + 1]
                            ),
                        )
                        # decay multiply (no normalization here): DVE
                        # tensor_tensor in 2x mode; negative-stride view
                        # of the Toeplitz decay tile.
                        hi = 127 + 128 * tq - p0
                        stop_i = hi - pl
                        din = (
                            dd_sb[:, hi:stop_i:-1]
                            if stop_i >= 0
                            else dd_sb[:, hi::-1]
                        )
                        nc.vector.tensor_tensor(
                            out=p_t[:, p0 : p0 + pl],
                            in0=p_t[:, p0 : p0 + pl],
                            in1=din,
                            op=Alu.mult,
                        )
                        k0b, k1b = p0 // 128, (p0 + pl) // 128
                        nc.sync.dma_start_transpose(
                            out=pdt[:, k0b:k1b, :],
                            in_=p_t[:, p0 : p0 + pl],
                        )
                    if npieces > 1:
                        nc.vector.reduce_sum(
                            out=z4[:, head : head + 1],
                            in_=zparts[:, 0:npieces],
                            axis=mybir.AxisListType.X,
                        )
                    return pdt

                def pv_stage(head, pdt):
                    pair, hin = head // 2, head % 2
                    prow = 64 * hin
                    for kc in range(tq + 1):
                        nc.tensor.matmul(
                            ps_y_t[pair][prow : prow + 64, 0:128],
                            lhsT=v_sb[:, kc, 64 * head : 64 * head + 64],
                            rhs=pdt[:, kc, :],
                            start=(kc == 0),
                            stop=(kc == tq),
                            tile_position=(0, prow),
                        )

                # software-pipelined: PE runs the next head's S matmuls
                # while the previous head's transpose is in flight.
                pdt0 = s_stage(0)
                pdt1 = s_stage(1)
                pv_stage(0, pdt0)
                pdt2 = s_stage(2)
                pv_stage(1, pdt1)
                pdt3 = s_stage(3)
                pv_stage(2, pdt2)
                pv_stage(3, pdt3)
                # ---- deferred softmax normalization ----
                zT = ps_y_t[0][0:4, 256:384]  # [4, 128] fp32 in PSUM
                nc.tensor.transpose(out=zT, in_=z4, identity=ident_sb)
                zT_sb = z_pool.tile([4, 128], fp32, tag="zTs")
                nc.vector.tensor_copy(out=zT_sb, in_=zT)
                rzT32 = z_pool.tile([4, 128], fp32, tag="rzT32")
                nc.vector.reciprocal(out=rzT32, in_=zT_sb)
                rzT = z_pool.tile([4, 128], bf16, tag="rzT")
                nc.vector.tensor_copy(out=rzT, in_=rzT32)
                for pair in range(2):
                    mbc = ps_y_t[pair][:, 128:256]
                    nc.tensor.matmul(
                        mbc,
                        lhsT=sel_sb[:, 128 * pair : 128 * pair + 128],
                        rhs=rzT,
                        start=True,
                        stop=True,
                    )
                    # walrus rejects TensorTensor with two PSUM operands:
                    # stage Mbc through SBUF (Pool), then stt (PSUM x SBUF).
                    mbc_sb = z_pool.tile([128, 128], bf16, tag="mbc")
                    nc.vector.tensor_copy(out=mbc_sb, in_=mbc)
                    nc.vector.scalar_tensor_tensor(
                        out=yt_sb[:, pair, 128 * tq : 128 * tq + 128],
                        in0=ps_y_t[pair][:, 0:128],
                        scalar=1.0,
                        in1=mbc_sb,
                        op0=Alu.mult,
                        op1=Alu.mult,
                    )
            def projection_tile(tq):
                o_t = out_pool.tile([128, C], fp32, tag="o")
                for nh in range(2):
                    ps_full = ps_s.tile([128, 1024], fp32, tag="s")
                    ps = ps_full[:, 0:512]
                    for pair in range(2):
                        nc.tensor.matmul(
                            ps,
                            lhsT=yt_sb[:, pair, 128 * tq : 128 * tq + 128],
                            rhs=wp_sb[:, pair, 512 * nh : 512 * nh + 512],
                            start=(pair == 0),
                            stop=(pair == 1),
                        )
                    cp = nc.vector.tensor_copy if nh == 0 else nc.scalar.copy
                    cp(out=o_t[:, 512 * nh : 512 * nh + 512], in_=ps)
                    nc.sync.dma_start(
                        out=out[128 * tq : 128 * tq + 128, 512 * nh : 512 * nh + 512],
                        in_=o_t[:, 512 * nh : 512 * nh + 512],
                    )

            # ---- schedule: K strips, then per quarter: Q chunk, V tiles,
            # attention tiles.  Engines overlap via tile-framework deps;
            # emitting attention early lets exp/decay/PV start while later
            # Q/V still stream through the PE.  Each tile's projection is
            # deferred one step so the PE isn't stalled on the softmax
            # normalization tail of the current tile. ----
            # attention tile tq only reads K columns < 128*(tq+1), so only
            # the first K chunk gates attn0; later K/Q chunks and V tiles
            # are spliced between attention tiles to keep ScalarE fed.
            nchunks = (T_ + 511) // 512
            qk_strip(1, kt_sb, 0)
            qk_strip(0, qt_sb, 0)
            nv = T_ // 128
            v_tile(0)
            v_next = 1
            for tq in range(NQ_):
                attention_tile(tq)
                if tq > 0:
                    projection_tile(tq - 1)
                if tq + 1 < nchunks:
                    qk_strip(1, kt_sb, tq + 1)
                if v_next < min(nv, tq + 2):
                    v_tile(v_next)
                    v_next += 1
                if tq % 4 == 1 and tq // 4 + 1 < nchunks:
                    qk_strip(0, qt_sb, tq // 4 + 1)
            projection_tile(NQ_ - 1)

    nc.compile()
    return nc


def make_in_maps(x, W_attn, W_proj, T_=T):
    """Host-side sharding: per-core input dicts."""
    x = np.asarray(x, dtype=np.float32)[:, :T_, :]
    W_attn = np.asarray(W_attn, dtype=np.float32)
    W_proj = np.asarray(W_proj, dtype=np.float32)

    dvals = np.zeros(2 * T_, dtype=np.float64)
    dvals[:T_] = _decay_values_np(T_)
    # dd[qi, m] = d(qi + m - 127), 0 where qi + m < 127
    qi = np.arange(128)[:, None]
    m = np.arange(T_)[None, :]
    idx = qi + m - 127
    dd = np.where(idx >= 0, dvals[np.clip(idx, 0, 2 * T_ - 1)], 0.0).astype(BF16)

    maskA = (
        (np.arange(128)[:, None] > np.arange(128)[None, :]).astype(np.float32)
    ).astype(BF16)  # strict lower ones: A[j, q] = 1 iff j > q
    negI = (-1e30 * np.eye(128, dtype=np.float32)).astype(BF16)
    ident = np.eye(128, dtype=np.float32)
    sel = np.zeros((4, 256), dtype=np.float32)
    for p in range(2):
        for mm in range(128):
            sel[2 * p + mm // 64, 128 * p + mm] = 1.0
    sel = sel.astype(BF16)

    in_maps = []
    for core in range(N_CORES):
        b = core // 4
        g = core % 4
        h0 = HEADS_PER_CORE * g  # first head of this core within the batch
        xT_c = np.ascontiguousarray(x[b].T).astype(BF16)  # [C, T]
        cols = []
        for which in range(2):  # q, k
            base = 1024 * which
            for pair in range(2):
                h = h0 + 2 * pair
                cols.append(W_attn[:, base + 64 * h : base + 64 * (h + 2)])
        cols.append(W_attn[:, 2048 + 64 * h0 : 2048 + 64 * (h0 + 4)])  # v
        wqkv_c = np.concatenate(cols, axis=1).astype(BF16)  # [C, 768]
        wp_c = W_proj[64 * h0 : 64 * (h0 + 4), :].astype(BF16)  # [256, C]
        in_maps.append(
            {
                "xT": xT_c,
                "wqkv": wqkv_c,
                "wp": wp_c,
                "dd": dd,
                "maskA": maskA,
                "negI": negI,
                "ident": ident,
                "sel": sel,
            }
        )
    return in_maps


def kernel(x, W_attn, W_proj):
    from concourse.bass_utils import run_bass_kernel_spmd

    in_maps = make_in_maps(x, W_attn, W_proj)
    nc = build_nc()
    res = run_bass_kernel_spmd(nc, in_maps, core_ids=list(range(N_CORES)))
    outs = [np.asarray(r["out"], dtype=np.float32) for r in res.results]
    full = np.zeros((B, T, C), dtype=np.float32)
    for core in range(N_CORES):
        full[core // 4] += outs[core]
    return full



# revision 6
# speedup vs baseline: 1.1668x; 1.1668x over previous
"""Causal self-attention with post-softmax decay mask — Trainium2 Bass kernel.

Problem shapes (hardcoded): B=2, T=2048, C=1024, H=16 heads, head_dim=64.
Sharding: 8 cores = (batch b = core//4) x (head group g = core%4, 4 heads each).
Each core computes QKV projection for its 4 heads from x[b], causal
flash-style attention with the decay mask, and a partial output projection
(its heads' 256 features x W_proj rows). Host sums the 4 partials per batch.

Per-core kernel (all matmuls bf16, fp32 PSUM accumulation):
  phase 1: K^T/Q^T strips and V^T strips from xT and the weight slice; V^T
           is flipped to [t, feat] layout with one merged XBAR transpose per
           512-t chunk per 128-feat pass (8 calls total) — cheaper than the
           per-tile [t,feat] matmul orientation (64 vs 128 PE matmuls).
  phase 2: per q-tile (128 rows) per head: S = Q K^T (row-packed matmul
           pairs), causal mask folded into the PSUM accumulation as an
           extra matmul (strict-lower ones^T @ -1e30*I), exp via ScalarE
           (scale=1/8) with accum_out giving row sums Z.  Softmax
           normalization is DEFERRED to the y^T tile.  exp pieces for all
           4 heads share one SBUF tile [128, 4, 1024] so a single XBAR
           transpose per (q-tile, piece) moves P^T for every head (24
           dispatches instead of 96 — the dispatch is ~1.2us fixed).
           Decay is a DVE tensor_tensor over 2 heads at once (negative-
           stride Toeplitz read from a 2-replica decay tile).
  phase 3: PV runs per GROUP of 2 q-tiles with F=256 matmuls: k-blocks
           kc <= 2g feed both tiles in one matmul (block 2g is exactly
           tile 2g's diagonal and a valid sub-diagonal block for 2g+1);
           one F=128 matmul adds tile 2g+1's diagonal.  Z for the group
           is reciprocated in [128,4] layout (full DVE lanes), PE-
           transposed to [4, 256], broadcast with a K=4 selector matmul,
           and applied on the PSUM->SBUF copy (scalar_tensor_tensor).
  phase 4: projection per q-tile (2 pair-accumulated F=512 matmuls per
           512-feature half) with one [128,1024] output DMA per q-tile.
"""

import math
import sys

sys.path.insert(0, "/opt/trn_rl_repo")

import numpy as np
import ml_dtypes

B, T, C = 2, 2048, 1024
N_HEAD = 16
HD = 64
HEADS_PER_CORE = 4
N_CORES = 8
NQ = T // 128  # 16 q-tiles

BF16 = ml_dtypes.bfloat16


def _decay_values_np(n):
    """decay_values[i] = decay at distance i, faithful to reference (fp64)."""
    dl = 2048 - 16 + 1
    nums = np.linspace(0.0, 1.0, dl, dtype=np.float64)
    decay_values = 1.0 - np.power(nums, 1.0 / np.e)
    return np.concatenate([np.ones(15), decay_values])[:n]


def _decay_matrix_np(n):
    """tril decay matrix, faithful to reference.decay_weight_matrix (fp32)."""
    decay_values = _decay_values_np(n)
    idx = np.arange(n)[:, None] - np.arange(n)[None, :]
    mat = decay_values[np.clip(idx, 0, n - 1)]
    return np.where(idx >= 0, mat, 0.0).astype(np.float32)


def build_nc(T_=T):
    import concourse.bass as bass
    import concourse.bacc as bacc
    import concourse.mybir as mybir
    import concourse.tile as tile

    fp32 = mybir.dt.float32
    bf16 = mybir.dt.bfloat16
    Alu = mybir.AluOpType
    Act = mybir.ActivationFunctionType

    NQ_ = T_ // 128
    NG_ = NQ_ // 2  # q-tile groups of 2
    PIECE = min(1024, T_)
    NP_ = (T_ + PIECE - 1) // PIECE  # max pieces per q-tile row
    KB_ = PIECE // 128  # k-blocks per piece
    nc = bacc.Bacc("TRN2")

    xT = nc.declare_dram_parameter("xT", [C, T_], bf16, isOutput=False)
    # wqkv columns: [q01(128) q23(128) k01(128) k23(128) v0..v3(256)]
    wqkv = nc.declare_dram_parameter("wqkv", [C, 768], bf16, isOutput=False)
    # wp rows: h0 feats(64), h1, h2, h3
    wp = nc.declare_dram_parameter("wp", [256, C], bf16, isOutput=False)
    # Toeplitz decay, replicated twice for 2-head batched DVE multiply:
    # dd4[qi, r, m] = d(qi + m - 127); blocks read columns with stride -1.
    dd4 = nc.declare_dram_parameter("dd4", [128, 2, T_], bf16, isOutput=False)
    # causal mask as a rank-128 matmul: maskA strict-lower ones, negI =
    # -1e30 * I; S_psum += maskA^T @ negI puts -1e30 above the diagonal.
    maskA = nc.declare_dram_parameter("maskA", [128, 128], bf16, isOutput=False)
    negI = nc.declare_dram_parameter("negI", [128, 128], bf16, isOutput=False)
    # fp32 identity for the PE transpose of 1/Z ([128,4] -> [4,128])
    ident = nc.declare_dram_parameter("ident", [128, 128], fp32, isOutput=False)
    # selector for broadcasting rzT rows to head-halves: sel[j, 128p + m]
    # = 1 iff j == 2p + m//64
    sel = nc.declare_dram_parameter("sel", [4, 256], bf16, isOutput=False)
    out = nc.declare_dram_parameter("out", [T_, C], fp32, isOutput=True)

    with tile.TileContext(nc) as tc:
        with (
            tc.tile_pool(name="const", bufs=1) as const_pool,
            tc.tile_pool(name="qkvout", bufs=1) as qkv_pool,
            tc.tile_pool(name="p", bufs=3) as p_pool,
            tc.tile_pool(name="pdt", bufs=2) as pdt_pool,
            tc.tile_pool(name="z", bufs=8) as z_pool,
            tc.tile_pool(name="outs", bufs=2) as out_pool,
            tc.tile_pool(name="ps_s", bufs=2, space="PSUM") as ps_s,
            tc.tile_pool(name="ps_y", bufs=3, space="PSUM") as ps_y_pool,
            tc.tile_pool(name="ps_n", bufs=1, space="PSUM") as ps_n_pool,
        ):
            # ---- load constants / inputs ----
            # xt/wqkv stream in kc-chunks so the first QKV matmul can start
            # after chunk 0 lands instead of after the full 6 MiB load.
            xt_sb = const_pool.tile([128, 8, T_], bf16)
            wqkv_sb = const_pool.tile([128, 8, 768], bf16)
            xT_r = xT.rearrange("(kc p) t -> p kc t", p=128)
            wqkv_r = wqkv.rearrange("(kc p) m -> p kc m", p=128)
            for kc in range(8):
                nc.sync.dma_start(
                    out=wqkv_sb[:, kc : kc + 1, :], in_=wqkv_r[:, kc : kc + 1, :]
                )
                nc.sync.dma_start(
                    out=xt_sb[:, kc : kc + 1, :], in_=xT_r[:, kc : kc + 1, :]
                )
            wp_sb = const_pool.tile([128, 2, C], bf16)
            nc.sync.dma_start(out=wp_sb, in_=wp.rearrange("(pr p) n -> p pr n", p=128))
            dd4_sb = const_pool.tile([128, 2, T_], bf16)
            nc.sync.dma_start(out=dd4_sb, in_=dd4[:, :, :])
            maskA_sb = const_pool.tile([128, 128], bf16)
            nc.sync.dma_start(out=maskA_sb, in_=maskA[:, :])
            negI_sb = const_pool.tile([128, 128], bf16)
            nc.sync.dma_start(out=negI_sb, in_=negI[:, :])
            ident_sb = const_pool.tile([128, 128], fp32)
            nc.sync.dma_start(out=ident_sb, in_=ident[:, :])
            sel_sb = const_pool.tile([4, 256], bf16)
            nc.sync.dma_start(out=sel_sb, in_=sel[:, :])

            qt_sb = qkv_pool.tile([128, 2, T_], bf16)  # [feat(2x64), pair, q]
            kt_sb = qkv_pool.tile([128, 2, T_], bf16)
            vt_sb = qkv_pool.tile([128, 2, T_], bf16)  # v^T strips [feat pass, t]
            v_sb = qkv_pool.tile([128, T_ // 128, 256], bf16)  # [k-rows, kc, 4 heads]
            yt_sb = qkv_pool.tile([128, 2, T_], bf16)  # y^T strips per pair

            cp_flip = [0]

            def cp(dst, src):
                """PSUM->SBUF copies alternate Vector/Scalar."""
                e = nc.vector.tensor_copy if cp_flip[0] % 2 == 0 else nc.scalar.copy
                cp_flip[0] += 1
                e(out=dst, in_=src)

            # ---- phase 1: QKV projections ----
            def qk_strip(which, dst, nqc):
                cw = min(512, T_ - 512 * nqc)
                for pair in range(2):
                    wcol = 256 * which + 128 * pair
                    ps_full = ps_s.tile([128, 1024], fp32, tag="s")
                    ps = ps_full[:, 0:512]
                    for kc in range(8):
                        nc.tensor.matmul(
                            ps[:, 0:cw],
                            lhsT=wqkv_sb[:, kc, wcol : wcol + 128],
                            rhs=xt_sb[:, kc, 512 * nqc : 512 * nqc + cw],
                            start=(kc == 0),
                            stop=(kc == 7),
                        )
                    cp(dst[:, pair, 512 * nqc : 512 * nqc + cw], ps[:, 0:cw])

            def vt_chunk(nvc):
                cw = min(512, T_ - 512 * nvc)
                for p in range(2):
                    ps_full = ps_s.tile([128, 1024], fp32, tag="s")
                    ps = ps_full[:, 0:512]
                    for kc in range(8):
                        nc.tensor.matmul(
                            ps[:, 0:cw],
                            lhsT=wqkv_sb[:, kc, 512 + 128 * p : 640 + 128 * p],
                            rhs=xt_sb[:, kc, 512 * nvc : 512 * nvc + cw],
                            start=(kc == 0),
                            stop=(kc == 7),
                        )
                    cp(vt_sb[:, p, 512 * nvc : 512 * nvc + cw], ps[:, 0:cw])
                    # flip v^T -> v[t, feat] for the PV lhsT (one XBAR call
                    # per (chunk, pass) moves cw/128 k-blocks at once)
                    nc.sync.dma_start_transpose(
                        out=v_sb[:, 4 * nvc : 4 * nvc + cw // 128, 128 * p : 128 * p + 128],
                        in_=vt_sb[:, p, 512 * nvc : 512 * nvc + cw],
                    )

            # ---- phase 2: S/exp/decay/transpose per q-tile ----
            def s_tile(tq, pdt):
                """All 4 heads: S matmuls + mask + exp + decay; one merged
                P^T transpose per piece.  Returns z4 [128, 4] (row sums)."""
                L = 128 * (tq + 1)
                tqi = tq % 2
                z4 = z_pool.tile([128, 4], fp32, tag="z4")
                npieces = (L + PIECE - 1) // PIECE
                zparts = None
                if npieces > 1:
                    zparts = z_pool.tile([128, 4, NP_], fp32, tag="zp")
                for piece in range(npieces):
                    p0 = PIECE * piece
                    pl = min(PIECE, L - p0)
                    p_t = p_pool.tile([128, 4, PIECE], bf16, tag="p")
                    if pl < PIECE:
                        # zero the tail so the merged transpose reads
                        # initialized data (k-blocks beyond L are never
                        # consumed by PV, but keep them deterministic).
                        # GpSimd is idle; keep this off Vector/Scalar.
                        nc.gpsimd.memset(p_t[:, :, pl:PIECE], 0)
                    for head in range(4):
                        pair, hin = head // 2, head % 2
                        prow = 64 * hin
                        ps_full = ps_s.tile([128, 1024], fp32, tag="s")
                        ps = ps_full[:, 0:PIECE]
                        nsc = (pl + 511) // 512
                        for sc in range(nsc):
                            scl = min(512, pl - 512 * sc)
                            k0 = p0 + 512 * sc
                            nc.tensor.matmul(
                                ps[:, 512 * sc : 512 * sc + scl],
                                lhsT=qt_sb[
                                    prow : prow + 64,
                                    pair,
                                    128 * tq : 128 * tq + 128,
                                ],
                                rhs=kt_sb[prow : prow + 64, pair, k0 : k0 + scl],
                                start=True,
                                stop=not (sc == nsc - 1 and p0 + pl == L),
                                tile_position=(prow, 0),
                            )
                        if p0 + pl == L:  # diagonal chunk lives here:
                            # accumulate the causal mask into PSUM via
                            # matmul: ps[:, diag] += maskA^T @ negI.
                            off = (L - 128) - p0
                            nc.tensor.matmul(
                                ps[:, off : off + 128],
                                lhsT=maskA_sb,
                                rhs=negI_sb,
                                start=False,
                                stop=True,
                            )
                        nc.scalar.activation(
                            out=p_t[:, head, 0:pl],
                            in_=ps[:, 0:pl],
                            func=Act.Exp,
                            scale=0.125,
                            accum_out=(
                                z4[:, head : head + 1]
                                if npieces == 1
                                else zparts[:, head, piece : piece + 1]
                            ),
                        )
                    # decay multiply (no normalization here): DVE
                    # tensor_tensor in 2x mode, 2 heads per op; negative-
                    # stride view of the 2-replica Toeplitz decay tile.
                    hi = 127 + 128 * tq - p0
                    stop_i = hi - pl
                    din = (
                        dd4_sb[:, :, hi:stop_i:-1]
                        if stop_i >= 0
                        else dd4_sb[:, :, hi::-1]
                    )
                    for hp in range(2):
                        nc.vector.tensor_tensor(
                            out=p_t[:, 2 * hp : 2 * hp + 2, 0:pl],
                            in0=p_t[:, 2 * hp : 2 * hp + 2, 0:pl],
                            in1=din,
                            op=Alu.mult,
                        )
                    # one XBAR transpose moves P^T for all 4 heads; k-blocks
                    # beyond this tile's L get garbage that PV never reads.
                    nc.sync.dma_start_transpose(
                        out=pdt[:, piece, :, :, tqi, :],
                        in_=p_t[:, :, :],
                    )
                if npieces > 1:
                    nc.vector.reduce_sum(
                        out=z4[:, :],
                        in_=zparts[:, :, 0:npieces],
                        axis=mybir.AxisListType.X,
                    )
                return z4

            # ---- phase 3: grouped PV + deferred softmax normalization ----
            def pv_norm_group(g, pdt, z4_pair):
                """PV for q-tiles (2g, 2g+1) with F=256 matmuls, then
                normalize into yt_sb."""
                ps_yg = ps_y_pool.tile([128, 512], fp32, tag="y")
                for head in range(4):
                    pair, hin = head // 2, head % 2
                    prow = 64 * hin
                    c0 = 256 * pair
                    for kc in range(2 * g + 1):  # feeds BOTH tiles (F=256)
                        nc.tensor.matmul(
                            ps_yg[prow : prow + 64, c0 : c0 + 256],
                            lhsT=v_sb[:, kc, 64 * head : 64 * head + 64],
                            rhs=pdt[:, kc // KB_, head, kc % KB_, :, :],
                            start=(kc == 0),
                            stop=False,
                            tile_position=(0, prow),
                        )
                    kc = 2 * g + 1  # diagonal of tile 2g+1 (F=128)
                    nc.tensor.matmul(
                        ps_yg[prow : prow + 64, c0 + 128 : c0 + 256],
                        lhsT=v_sb[:, kc, 64 * head : 64 * head + 64],
                        rhs=pdt[:, kc // KB_, head, kc % KB_, 1, :],
                        start=False,
                        stop=True,
                        tile_position=(0, prow),
                    )
                # 1/Z in [128, 4] layout (full DVE lanes), then PE-transpose
                rz = [None, None]
                for tqi in range(2):
                    rz4 = z_pool.tile([128, 4], fp32, tag="rz4")
                    nc.vector.reciprocal(out=rz4, in_=z4_pair[tqi])
                    rz[tqi] = rz4
                zT = ps_n_pool.tile([128, 256], fp32, tag="n")
                for tqi in range(2):
                    nc.tensor.transpose(
                        out=zT[0:4, 128 * tqi : 128 * tqi + 128],
                        in_=rz[tqi],
                        identity=ident_sb,
                    )
                rz_sb = z_pool.tile([4, 256], bf16, tag="rzs")
                nc.vector.tensor_copy(out=rz_sb, in_=zT[0:4, :])
                for pair in range(2):
                    mbc = ps_n_pool.tile([128, 256], fp32, tag="n")
                    nc.tensor.matmul(
                        mbc,
                        lhsT=sel_sb[:, 128 * pair : 128 * pair + 128],
                        rhs=rz_sb,
                        start=True,
                        stop=True,
                    )
                    # walrus rejects TensorTensor with two PSUM operands:
                    # stage Mbc through SBUF, then stt (PSUM x SBUF).
                    mbc_sb = z_pool.tile([128, 256], bf16, tag="mbcs")
                    nc.vector.tensor_copy(out=mbc_sb, in_=mbc)
                    nc.vector.scalar_tensor_tensor(
                        out=yt_sb[:, pair, 256 * g : 256 * g + 256],
                        in0=ps_yg[:, 256 * pair : 256 * pair + 256],
                        scalar=1.0,
                        in1=mbc_sb,
                        op0=Alu.mult,
                        op1=Alu.mult,
                    )

            def projection_tile(tq):
                o_t = out_pool.tile([128, C], fp32, tag="o")
                for nh in range(2):
                    ps = ps_y_pool.tile([128, 512], fp32, tag="y")
                    for pair in range(2):
                        nc.tensor.matmul(
                            ps,
                            lhsT=yt_sb[:, pair, 128 * tq : 128 * tq + 128],
                            rhs=wp_sb[:, pair, 512 * nh : 512 * nh + 512],
                            start=(pair == 0),
                            stop=(pair == 1),
                        )
                    cp(o_t[:, 512 * nh : 512 * nh + 512], ps)
                nc.sync.dma_start(
                    out=out[128 * tq : 128 * tq + 128, :],
                    in_=o_t,
                )

            # ---- schedule ----
            nchunks = (T_ + 511) // 512
            qk_strip(1, kt_sb, 0)
            qk_strip(0, qt_sb, 0)
            vt_chunk(0)
            for g in range(NG_):
                pdt = pdt_pool.tile([128, NP_, 4, KB_, 2, 128], bf16, tag="pdt")
                z4a = s_tile(2 * g, pdt)
                if g > 0:
                    projection_tile(2 * g - 2)
                    projection_tile(2 * g - 1)
                z4b = s_tile(2 * g + 1, pdt)
                if g + 1 < nchunks:
                    qk_strip(1, kt_sb, g + 1)
                    vt_chunk(g + 1)
                if g % 2 == 0 and g // 2 + 1 < nchunks:
                    qk_strip(0, qt_sb, g // 2 + 1)
                pv_norm_group(g, pdt, (z4a, z4b))
            projection_tile(NQ_ - 2)
            projection_tile(NQ_ - 1)

    nc.compile()
    return nc


def make_in_maps(x, W_attn, W_proj, T_=T):
    """Host-side sharding: per-core input dicts."""
    x = np.asarray(x, dtype=np.float32)[:, :T_, :]
    W_attn = np.asarray(W_attn, dtype=np.float32)
    W_proj = np.asarray(W_proj, dtype=np.float32)

    dvals = np.zeros(2 * T_, dtype=np.float64)
    dvals[:T_] = _decay_values_np(T_)
    # dd[qi, m] = d(qi + m - 127), 0 where qi + m < 127
    qi = np.arange(128)[:, None]
    m = np.arange(T_)[None, :]
    idx = qi + m - 127
    dd = np.where(idx >= 0, dvals[np.clip(idx, 0, 2 * T_ - 1)], 0.0).astype(BF16)
    dd4 = np.ascontiguousarray(np.stack([dd, dd], axis=1))  # [128, 2, T_]

    maskA = (
        (np.arange(128)[:, None] > np.arange(128)[None, :]).astype(np.float32)
    ).astype(BF16)  # strict lower ones: A[j, q] = 1 iff j > q
    negI = (-1e30 * np.eye(128, dtype=np.float32)).astype(BF16)
    ident = np.eye(128, dtype=np.float32)
    sel = np.zeros((4, 256), dtype=np.float32)
    for p in range(2):
        for mm in range(128):
            sel[2 * p + mm // 64, 128 * p + mm] = 1.0
    sel = sel.astype(BF16)

    in_maps = []
    for core in range(N_CORES):
        b = core // 4
        g = core % 4
        h0 = HEADS_PER_CORE * g  # first head of this core within the batch
        xT_c = np.ascontiguousarray(x[b].T).astype(BF16)  # [C, T]
        cols = []
        for which in range(2):  # q, k
            base = 1024 * which
            for pair in range(2):
                h = h0 + 2 * pair
                cols.append(W_attn[:, base + 64 * h : base + 64 * (h + 2)])
        cols.append(W_attn[:, 2048 + 64 * h0 : 2048 + 64 * (h0 + 4)])  # v
        wqkv_c = np.concatenate(cols, axis=1).astype(BF16)  # [C, 768]
        wp_c = W_proj[64 * h0 : 64 * (h0 + 4), :].astype(BF16)  # [256, C]
        in_maps.append(
            {
                "xT": xT_c,
                "wqkv": wqkv_c,
                "wp": wp_c,
                "dd4": dd4,
                "maskA": maskA,
                "negI": negI,
                "ident": ident,
                "sel": sel,
            }
        )
    return in_maps


def kernel(x, W_attn, W_proj):
    from concourse.bass_utils import run_bass_kernel_spmd

    in_maps = make_in_maps(x, W_attn, W_proj)
    nc = build_nc()
    res = run_bass_kernel_spmd(nc, in_maps, core_ids=list(range(N_CORES)))
    outs = [np.asarray(r["out"], dtype=np.float32) for r in res.results]
    full = np.zeros((B, T, C), dtype=np.float32)
    for core in range(N_CORES):
        full[core // 4] += outs[core]
    return full


# revision 8
# speedup vs baseline: 1.1942x; 1.0235x over previous
"""Causal self-attention with post-softmax decay mask — Trainium2 Bass kernel.

Problem shapes (hardcoded): B=2, T=2048, C=1024, H=16 heads, head_dim=64.
Sharding: 8 cores = (batch b = core//4) x (head group g = core%4, 4 heads each).
Each core computes QKV projection for its 4 heads from x[b], causal
flash-style attention with the decay mask, and a partial output projection
(its heads' 256 features x W_proj rows). Host sums the 4 partials per batch.

Key structure (v2 — transpose-free attention):
  The attention probabilities are computed directly in TRANSPOSED
  orientation, S^T[k, q] = (K Q^T), with k on partitions — so the PV
  contraction (over k) needs no XBAR transpose of P at all (the per-block
  transpose dispatch cost ~160ns x 544 blocks dominated earlier versions).

  Per GROUP of 2 q-tiles (q columns 256g..256g+255), per head:
    - S^T k-blocks (F=256 matmuls, K=64, head pairs packed into PE row
      halves via tile_position).  Blocks are laid out in DESCENDING kc
      order; block kcd=0 is q-tile 2g+1's diagonal (F=128, only the
      tqi=1 half exists).  Causal mask = extra matmul negI^T @ maskA
      accumulated into the diagonal blocks' PSUM.
    - exp via ScalarE (scale=1/8) into an SBUF tile pT [128, kcd, 2, 128].
    - Z row sums via PE: ones32^T @ pT per block, m=32, accumulated into
      a 32-row band of one PSUM tile per head (4 heads -> 4 distinct PE
      column groups, running concurrently, tile fully covered so the
      reciprocal reads no uninitialized rows).  Z uses the UNdecayed exp
      (reference normalizes before the multiplicative decay).
    - decay: DVE tensor_tensor, ONE op for all sub-diagonal blocks via a
      Toeplitz tile ddk2[k, r, m] = d(m + 128 r - k) (the replica dim r
      doubles as the q-tile-within-group shift) + one diagonal-block op.
      d(m-k)=0 for m<k doubles as causal zeroing.  Runs after the Z
      matmuls (in-place).
    - PV: F=256 matmuls over kc (both q-tiles at once; block kc=2g is
      tile 2g's diagonal AND a valid block for 2g+1) + one F=128 for
      tile 2g+1's diagonal.
    - normalization: reciprocal of the banded Z tile, selector matmul
      broadcast (selP, K=128) to head-halves, applied on the PSUM->SBUF
      copy (scalar_tensor_tensor).
  V is produced as V^T strips (F=512 matmuls) and flipped to [t, feat]
  with one XBAR transpose per (512-t chunk, 128-feat pass) — 8 calls.
  Projection per q-tile: 2 pair-accumulated F=512 matmuls per 512-feature
  half, one [128,1024] output DMA per q-tile.
"""

import math
import sys

sys.path.insert(0, "/opt/trn_rl_repo")

import numpy as np
import ml_dtypes

B, T, C = 2, 2048, 1024
N_HEAD = 16
HD = 64
HEADS_PER_CORE = 4
N_CORES = 8
NQ = T // 128  # 16 q-tiles

BF16 = ml_dtypes.bfloat16


def _decay_values_np(n):
    """decay_values[i] = decay at distance i, faithful to reference (fp64)."""
    dl = 2048 - 16 + 1
    nums = np.linspace(0.0, 1.0, dl, dtype=np.float64)
    decay_values = 1.0 - np.power(nums, 1.0 / np.e)
    return np.concatenate([np.ones(15), decay_values])[:n]


def _decay_matrix_np(n):
    """tril decay matrix, faithful to reference.decay_weight_matrix (fp32)."""
    decay_values = _decay_values_np(n)
    idx = np.arange(n)[:, None] - np.arange(n)[None, :]
    mat = decay_values[np.clip(idx, 0, n - 1)]
    return np.where(idx >= 0, mat, 0.0).astype(np.float32)


def build_nc(T_=T):
    import concourse.bass as bass
    import concourse.bacc as bacc
    import concourse.mybir as mybir
    import concourse.tile as tile

    fp32 = mybir.dt.float32
    bf16 = mybir.dt.bfloat16
    Alu = mybir.AluOpType
    Act = mybir.ActivationFunctionType

    NQ_ = T_ // 128
    NG_ = NQ_ // 2  # q-tile groups of 2
    nc = bacc.Bacc("TRN2")

    xT = nc.declare_dram_parameter("xT", [C, T_], bf16, isOutput=False)
    # wqkv columns: [q01(128) q23(128) k01(128) k23(128) v0..v3(256)]
    wqkv = nc.declare_dram_parameter("wqkv", [C, 768], bf16, isOutput=False)
    # wp rows: h0 feats(64), h1, h2, h3
    wp = nc.declare_dram_parameter("wp", [256, C], bf16, isOutput=False)
    # Toeplitz decay: ddk2[k, r, m] = d(m + 128r - k), 0 where m + 128r < k.
    ddk2 = nc.declare_dram_parameter("ddk2", [128, 2, T_], bf16, isOutput=False)
    # causal mask in S^T orientation: ps[k, q] += negI^T @ maskA =
    # -1e30 where k > q (maskA[j, q] = 1 iff j > q).
    maskA = nc.declare_dram_parameter("maskA", [128, 128], bf16, isOutput=False)
    negI = nc.declare_dram_parameter("negI", [128, 128], bf16, isOutput=False)
    # Z row-sum matmul weights (all ones, m=32 band)
    ones32 = nc.declare_dram_parameter("ones32", [128, 32], bf16, isOutput=False)
    # selector broadcast: selP[j, 128*pair + p] = 1 iff j = 64*pair + 32*(p//64)
    selP = nc.declare_dram_parameter("selP", [128, 256], bf16, isOutput=False)
    out = nc.declare_dram_parameter("out", [T_, C], fp32, isOutput=True)

    with tile.TileContext(nc) as tc:
        with (
            tc.tile_pool(name="const", bufs=1) as const_pool,
            tc.tile_pool(name="qkvout", bufs=1) as qkv_pool,
            tc.tile_pool(name="p", bufs=2) as p_pool,
            tc.tile_pool(name="z", bufs=8) as z_pool,
            tc.tile_pool(name="outs", bufs=2) as out_pool,
            tc.tile_pool(name="ps_s", bufs=2, space="PSUM") as ps_s,
            tc.tile_pool(name="ps_y", bufs=2, space="PSUM") as ps_y_pool,
            tc.tile_pool(name="ps_z", bufs=1, space="PSUM") as ps_z_pool,
            tc.tile_pool(name="ps_n", bufs=1, space="PSUM") as ps_n_pool,
        ):
            # ---- load constants / inputs ----
            xt_sb = const_pool.tile([128, 8, T_], bf16)
            wqkv_sb = const_pool.tile([128, 8, 768], bf16)
            xT_r = xT.rearrange("(kc p) t -> p kc t", p=128)
            wqkv_r = wqkv.rearrange("(kc p) m -> p kc m", p=128)
            for kc in range(8):
                nc.sync.dma_start(
                    out=wqkv_sb[:, kc : kc + 1, :], in_=wqkv_r[:, kc : kc + 1, :]
                )
                nc.sync.dma_start(
                    out=xt_sb[:, kc : kc + 1, :], in_=xT_r[:, kc : kc + 1, :]
                )
            wp_sb = const_pool.tile([128, 2, C], bf16)
            nc.sync.dma_start(out=wp_sb, in_=wp.rearrange("(pr p) n -> p pr n", p=128))
            ddk2_sb = const_pool.tile([128, 2, T_], bf16)
            nc.sync.dma_start(out=ddk2_sb, in_=ddk2[:, :, :])
            # [k, kcd, tqi, q] view: d(128*(kcd + tqi) + q - k)
            ddk2_r = ddk2_sb[:, :, :].rearrange("p r (a q) -> p a r q", q=128)
            maskA_sb = const_pool.tile([128, 128], bf16)
            nc.sync.dma_start(out=maskA_sb, in_=maskA[:, :])
            negI_sb = const_pool.tile([128, 128], bf16)
            nc.sync.dma_start(out=negI_sb, in_=negI[:, :])
            ones32_sb = const_pool.tile([128, 32], bf16)
            nc.sync.dma_start(out=ones32_sb, in_=ones32[:, :])
            selP_sb = const_pool.tile([128, 256], bf16)
            nc.sync.dma_start(out=selP_sb, in_=selP[:, :])

            qt_sb = qkv_pool.tile([128, 2, T_], bf16)  # [feat(2x64), pair, q]
            kt_sb = qkv_pool.tile([128, 2, T_], bf16)
            vt_sb = qkv_pool.tile([128, 2, T_], bf16)  # v^T strips [feat pass, t]
            v_sb = qkv_pool.tile([128, T_ // 128, 256], bf16)  # [k-rows, kc, 4 heads]
            yt_sb = qkv_pool.tile([128, 2, T_], bf16)  # y^T strips per pair

            cp_flip = [0]

            def cp(dst, src):
                """PSUM->SBUF copies alternate Vector/Scalar."""
                e = nc.vector.tensor_copy if cp_flip[0] % 2 == 0 else nc.scalar.copy
                cp_flip[0] += 1
                e(out=dst, in_=src)

            # ---- phase 1: QKV projections ----
            def qk_strip(which, dst, nqc):
                cw = min(512, T_ - 512 * nqc)
                for pair in range(2):
                    wcol = 256 * which + 128 * pair
                    ps_full = ps_s.tile([128, 1024], fp32, tag="s")
                    ps = ps_full[:, 0:512]
                    for kc in range(8):
                        nc.tensor.matmul(
                            ps[:, 0:cw],
                            lhsT=wqkv_sb[:, kc, wcol : wcol + 128],
                            rhs=xt_sb[:, kc, 512 * nqc : 512 * nqc + cw],
                            start=(kc == 0),
                            stop=(kc == 7),
                        )
                    cp(dst[:, pair, 512 * nqc : 512 * nqc + cw], ps[:, 0:cw])

            def vt_chunk(nvc):
                cw = min(512, T_ - 512 * nvc)
                for p in range(2):
                    ps_full = ps_s.tile([128, 1024], fp32, tag="s")
                    ps = ps_full[:, 0:512]
                    for kc in range(8):
                        nc.tensor.matmul(
                            ps[:, 0:cw],
                            lhsT=wqkv_sb[:, kc, 512 + 128 * p : 640 + 128 * p],
                            rhs=xt_sb[:, kc, 512 * nvc : 512 * nvc + cw],
                            start=(kc == 0),
                            stop=(kc == 7),
                        )
                    cp(vt_sb[:, p, 512 * nvc : 512 * nvc + cw], ps[:, 0:cw])
                    # flip v^T -> v[t, feat] for the PV lhsT
                    nc.sync.dma_start_transpose(
                        out=v_sb[
                            :, 4 * nvc : 4 * nvc + cw // 128, 128 * p : 128 * p + 128
                        ],
                        in_=vt_sb[:, p, 512 * nvc : 512 * nvc + cw],
                    )

            # ---- phase 2: S^T / exp / Z / decay per group ----
            def st_group(g):
                """S^T, exp, Z matmuls and decay for q-tiles (2g, 2g+1), all
                4 heads.  Returns (pT tiles, zps)."""
                nblk = 2 * g + 2  # k-blocks, kcd descending: kcd=0 <-> kc=2g+1
                npiece = (nblk + 3) // 4
                pTs = []
                for head in range(4):
                    pT = p_pool.tile([128, NQ_, 2, 128], bf16, tag=f"p{head}")
                    pTs.append(pT)
                zps = ps_z_pool.tile([128, 256], fp32, tag="z")
                q0 = 256 * g
                for piece in range(npiece):
                    j0 = 4 * piece
                    nb = min(4, nblk - j0)
                    for head in range(4):
                        pair, hin = head // 2, head % 2
                        prow = 64 * hin
                        pT = pTs[head]
                        pTf = pT[:, :, :, :].rearrange("p a b q -> p (a b q)")
                        ps_full = ps_s.tile([128, 1024], fp32, tag="s")
                        ps = ps_full
                        for jj in range(j0, j0 + nb):
                            kc = 2 * g + 1 - jj
                            c0 = 256 * (jj - j0)
                            lhsT = kt_sb[
                                prow : prow + 64, pair, 128 * kc : 128 * kc + 128
                            ]
                            if jj == 0:
                                # diagonal of tile 2g+1: only the tqi=1 half
                                nc.tensor.matmul(
                                    ps[:, c0 + 128 : c0 + 256],
                                    lhsT=lhsT,
                                    rhs=qt_sb[
                                        prow : prow + 64, pair, q0 + 128 : q0 + 256
                                    ],
                                    start=True,
                                    stop=False,
                                    tile_position=(prow, 0),
                                )
                                nc.tensor.matmul(
                                    ps[:, c0 + 128 : c0 + 256],
                                    lhsT=negI_sb,
                                    rhs=maskA_sb,
                                    start=False,
                                    stop=True,
                                )
                            elif jj == 1:
                                # kc=2g: diagonal of tile 2g (mask tqi=0 half)
                                nc.tensor.matmul(
                                    ps[:, c0 : c0 + 256],
                                    lhsT=lhsT,
                                    rhs=qt_sb[prow : prow + 64, pair, q0 : q0 + 256],
                                    start=True,
                                    stop=False,
                                    tile_position=(prow, 0),
                                )
                                nc.tensor.matmul(
                                    ps[:, c0 : c0 + 128],
                                    lhsT=negI_sb,
                                    rhs=maskA_sb,
                                    start=False,
                                    stop=True,
                                )
                            else:
                                nc.tensor.matmul(
                                    ps[:, c0 : c0 + 256],
                                    lhsT=lhsT,
                                    rhs=qt_sb[prow : prow + 64, pair, q0 : q0 + 256],
                                    start=True,
                                    stop=True,
                                    tile_position=(prow, 0),
                                )
                        if piece == 0:
                            # cols 0:128 (dead half of the kcd=0 block) were
                            # never written; exp the rest in one op.
                            nc.scalar.activation(
                                out=pTf[:, 128 : 256 * nb],
                                in_=ps[:, 128 : 256 * nb],
                                func=Act.Exp,
                                scale=0.125,
                            )
                        else:
                            nc.scalar.activation(
                                out=pTf[:, 256 * j0 : 256 * (j0 + nb)],
                                in_=ps[:, 0 : 256 * nb],
                                func=Act.Exp,
                                scale=0.125,
                            )
                    # Z partial sums for this piece (undecayed exp);
                    # the diagonal block (jj=0) is deferred to close the
                    # accumulation group.
                    for head in range(4):
                        for jj in range(max(1, j0), j0 + nb):
                            nc.tensor.matmul(
                                zps[32 * head : 32 * head + 32, 0:256],
                                lhsT=ones32_sb,
                                rhs=pTs[head][:, jj, :, :],
                                start=(jj == 1),
                                stop=False,
                                tile_position=(0, 32 * head),
                                skip_group_check=True,
                            )
                for head in range(4):
                    nc.tensor.matmul(
                        zps[32 * head : 32 * head + 32, 128:256],
                        lhsT=ones32_sb,
                        rhs=pTs[head][:, 0, 1, :],
                        start=False,
                        stop=True,
                        tile_position=(0, 32 * head),
                        skip_group_check=True,
                    )
                # decay (in place, after the Z sums): one op for all
                # sub-diagonal blocks + one for the 2g+1 diagonal.
                for head in range(4):
                    eng = nc.gpsimd if head == 3 else nc.vector
                    pT = pTs[head]
                    eng.tensor_tensor(
                        out=pT[:, 1:nblk, :, :],
                        in0=pT[:, 1:nblk, :, :],
                        in1=ddk2_r[:, 0 : nblk - 1, :, :],
                        op=Alu.mult,
                    )
                    eng.tensor_tensor(
                        out=pT[:, 0, 1, :],
                        in0=pT[:, 0, 1, :],
                        in1=ddk2_sb[:, 0, 0:128],
                        op=Alu.mult,
                    )
                return pTs, zps

            # ---- phase 3: grouped PV + deferred softmax normalization ----
            def pv_norm_group(g, pTs, zps):
                ps_yg = ps_y_pool.tile([128, 512], fp32, tag="y")
                for head in range(4):
                    pair, hin = head // 2, head % 2
                    prow = 64 * hin
                    c0 = 256 * pair
                    pT = pTs[head]
                    for kc in range(2 * g + 1):
                        nc.tensor.matmul(
                            ps_yg[prow : prow + 64, c0 : c0 + 256],
                            lhsT=v_sb[:, kc, 64 * head : 64 * head + 64],
                            rhs=pT[:, 2 * g + 1 - kc, :, :],
                            start=(kc == 0),
                            stop=False,
                            tile_position=(0, prow),
                        )
                    nc.tensor.matmul(
                        ps_yg[prow : prow + 64, c0 + 128 : c0 + 256],
                        lhsT=v_sb[:, 2 * g + 1, 64 * head : 64 * head + 64],
                        rhs=pT[:, 0, 1, :],
                        start=False,
                        stop=True,
                        tile_position=(0, prow),
                    )
                rz = z_pool.tile([128, 256], fp32, tag="rz")
                nc.vector.reciprocal(out=rz, in_=zps)
                rzb = z_pool.tile([128, 256], bf16, tag="rzb")
                nc.vector.tensor_copy(out=rzb, in_=rz)
                for pair in range(2):
                    mbc = ps_n_pool.tile([128, 256], fp32, tag="n")
                    nc.tensor.matmul(
                        mbc,
                        lhsT=selP_sb[:, 128 * pair : 128 * pair + 128],
                        rhs=rzb,
                        start=True,
                        stop=True,
                    )
                    mbc_sb = z_pool.tile([128, 256], bf16, tag="mbcs")
                    nc.vector.tensor_copy(out=mbc_sb, in_=mbc)
                    nc.vector.scalar_tensor_tensor(
                        out=yt_sb[:, pair, 256 * g : 256 * g + 256],
                        in0=ps_yg[:, 256 * pair : 256 * pair + 256],
                        scalar=1.0,
                        in1=mbc_sb,
                        op0=Alu.mult,
                        op1=Alu.mult,
                    )

            def projection_tile(tq):
                o_t = out_pool.tile([128, C], fp32, tag="o")
                for nh in range(2):
                    ps = ps_y_pool.tile([128, 512], fp32, tag="y")
                    for pair in range(2):
                        nc.tensor.matmul(
                            ps,
                            lhsT=yt_sb[:, pair, 128 * tq : 128 * tq + 128],
                            rhs=wp_sb[:, pair, 512 * nh : 512 * nh + 512],
                            start=(pair == 0),
                            stop=(pair == 1),
                        )
                    cp(o_t[:, 512 * nh : 512 * nh + 512], ps)
                nc.sync.dma_start(
                    out=out[128 * tq : 128 * tq + 128, :],
                    in_=o_t,
                )

            # ---- schedule ----
            nchunks = (T_ + 511) // 512
            qk_strip(1, kt_sb, 0)
            qk_strip(0, qt_sb, 0)
            vt_chunk(0)
            for g in range(NG_):
                pTs, zps = st_group(g)
                if g > 0:
                    projection_tile(2 * g - 2)
                    projection_tile(2 * g - 1)
                if g + 1 < nchunks:
                    qk_strip(1, kt_sb, g + 1)
                    vt_chunk(g + 1)
                if g % 2 == 0 and g // 2 + 1 < nchunks:
                    qk_strip(0, qt_sb, g // 2 + 1)
                pv_norm_group(g, pTs, zps)
            projection_tile(NQ_ - 2)
            projection_tile(NQ_ - 1)

    nc.compile()
    return nc


def make_in_maps(x, W_attn, W_proj, T_=T):
    """Host-side sharding: per-core input dicts."""
    x = np.asarray(x, dtype=np.float32)[:, :T_, :]
    W_attn = np.asarray(W_attn, dtype=np.float32)
    W_proj = np.asarray(W_proj, dtype=np.float32)

    dvals = np.zeros(2 * T_ + 256, dtype=np.float64)
    dvals[:T_] = _decay_values_np(T_)
    # ddk2[k, r, m] = d(m + 128r - k), 0 where m + 128r < k
    k = np.arange(128)[:, None, None]
    r = np.arange(2)[None, :, None]
    m = np.arange(T_)[None, None, :]
    idx = m + 128 * r - k
    ddk2 = np.where(idx >= 0, dvals[np.clip(idx, 0, 2 * T_ - 1)], 0.0).astype(BF16)
    ddk2 = np.ascontiguousarray(ddk2)

    maskA = (
        (np.arange(128)[:, None] > np.arange(128)[None, :]).astype(np.float32)
    ).astype(BF16)  # strict lower ones: maskA[j, q] = 1 iff j > q
    negI = (-1e30 * np.eye(128, dtype=np.float32)).astype(BF16)
    ones32 = np.ones((128, 32), dtype=np.float32).astype(BF16)
    selP = np.zeros((128, 256), dtype=np.float32)
    for pair in range(2):
        for p in range(128):
            selP[64 * pair + 32 * (p // 64), 128 * pair + p] = 1.0
    selP = selP.astype(BF16)

    in_maps = []
    for core in range(N_CORES):
        b = core // 4
        g = core % 4
        h0 = HEADS_PER_CORE * g  # first head of this core within the batch
        xT_c = np.ascontiguousarray(x[b].T).astype(BF16)  # [C, T]
        cols = []
        for which in range(2):  # q, k
            base = 1024 * which
            for pair in range(2):
                h = h0 + 2 * pair
                cols.append(W_attn[:, base + 64 * h : base + 64 * (h + 2)])
        cols.append(W_attn[:, 2048 + 64 * h0 : 2048 + 64 * (h0 + 4)])  # v
        wqkv_c = np.concatenate(cols, axis=1).astype(BF16)  # [C, 768]
        wp_c = W_proj[64 * h0 : 64 * (h0 + 4), :].astype(BF16)  # [256, C]
        in_maps.append(
            {
                "xT": xT_c,
                "wqkv": wqkv_c,
                "wp": wp_c,
                "ddk2": ddk2,
                "maskA": maskA,
                "negI": negI,
                "ones32": ones32,
                "selP": selP,
            }
        )
    return in_maps


def kernel(x, W_attn, W_proj):
    from concourse.bass_utils import run_bass_kernel_spmd

    in_maps = make_in_maps(x, W_attn, W_proj)
    nc = build_nc()
    res = run_bass_kernel_spmd(nc, in_maps, core_ids=list(range(N_CORES)))
    outs = [np.asarray(r["out"], dtype=np.float32) for r in res.results]
    full = np.zeros((B, T, C), dtype=np.float32)
    for core in range(N_CORES):
        full[core // 4] += outs[core]
    return full
